# revision 13
# baseline (speedup 1.0000x reference)
"""Trainium2 Bass kernel for causal self-attention with 2D RoPE.

Sharding: batch x head-group parallel over 8 NeuronCores.
  core c -> batch b = c // 4, heads h0 = (c % 4) * 3 .. h0+2.
Each core computes q/k/v projections for its 3 heads, 2D RoPE, causal
flash-attention (transposed-score layout, denominator via an appended
ones-column on V), and a per-head output projection with the softmax
normalization folded into the PSUM eviction scale. The host sums the
4 partial outputs per batch.

Matmuls run in bf16 (fp32 PSUM accumulation).
"""

import sys

sys.path.insert(0, "/opt/trn_rl_repo")

import numpy as np
from ml_dtypes import bfloat16

import concourse.bacc as bacc
import concourse.mybir as mybir
from concourse import tile
from concourse.bass_utils import run_bass_kernel_spmd

BF = mybir.dt.bfloat16
F32 = mybir.dt.float32
AF = mybir.ActivationFunctionType
ALU = mybir.AluOpType

P = 128          # partitions
DM = 768         # d_model
HD = 64          # head dim
NHC = 3          # heads per core
NCC = DM // P    # contraction chunks (6)
SQT = 512        # q-block (matmul moving dim)
QKV = 3 * NHC * HD  # 576
VW = NHC * (HD + 1)  # v + ones columns (195)


def build_program(S=2048, n_devices=8, trace=False):
    NS = S // P      # seq chunks of 128
    NQ = S // SQT    # q blocks of 512
    KPQ = SQT // P   # k-chunks per q-block (4)

    nc = bacc.Bacc(
        "TRN2", target_bir_lowering=False, debug=False, num_devices=n_devices
    )
    xt_d = nc.dram_tensor("xt", [DM, S], BF, kind="ExternalInput")
    wqkv_d = nc.dram_tensor("wqkv", [DM, QKV], BF, kind="ExternalInput")
    wo_d = nc.dram_tensor("wo", [NHC * HD, DM], BF, kind="ExternalInput")
    cos_d = nc.dram_tensor("cos", [S, NHC * HD], BF, kind="ExternalInput")
    sin_d = nc.dram_tensor("sin", [S, NHC * HD], BF, kind="ExternalInput")
    mask_d = nc.dram_tensor("masks", [P, KPQ, SQT], BF, kind="ExternalInput")
    id_d = nc.dram_tensor("ident", [P, P], BF, kind="ExternalInput")
    out_d = nc.dram_tensor("outp", [S, DM], F32, kind="ExternalOutput")

    with tile.TileContext(nc) as tc:
        with (
            tc.tile_pool(name="const", bufs=1) as const,
            tc.tile_pool(name="resid", bufs=1) as resid,
        ):
            xt_sb = const.tile([P, NCC, S], BF)
            nc.sync.dma_start(xt_sb[:], xt_d.rearrange("(n p) m -> p n m", p=P))
            wqkv_sb = const.tile([P, NCC, QKV], BF)
            nc.sync.dma_start(wqkv_sb[:], wqkv_d.rearrange("(n p) m -> p n m", p=P))
            wo_sb = const.tile([HD, NHC, DM], BF)
            for h in range(NHC):
                nc.sync.dma_start(wo_sb[:, h, :], wo_d[h * HD : (h + 1) * HD, :])
            cos_sb = const.tile([P, NS, NHC * HD], BF)
            nc.sync.dma_start(cos_sb[:], cos_d.rearrange("(n p) m -> p n m", p=P))
            sin_sb = const.tile([P, NS, NHC * HD], BF)
            nc.sync.dma_start(sin_sb[:], sin_d.rearrange("(n p) m -> p n m", p=P))
            mask_sb = const.tile([P, KPQ, SQT], BF)
            nc.sync.dma_start(mask_sb[:], mask_d[:])
            id_sb = const.tile([P, P], BF)
            nc.sync.dma_start(id_sb[:], id_d[:])

            q_sb = resid.tile([P, NS, NHC * HD], BF)
            k_sb = resid.tile([P, NS, NHC * HD], BF)
            v_sb = resid.tile([P, NS, VW], BF)
            qt01 = resid.tile([P, S], BF)
            qt2 = resid.tile([HD, S], BF)
            kt01 = resid.tile([P, S], BF)
            kt2 = resid.tile([HD, S], BF)
            ao = [resid.tile([HD + 1, S], BF, name=f"ao{h}") for h in range(NHC)]
            den_rec = resid.tile([P, NHC, NS], F32)

            # ones columns of v (denominator trick)
            v4 = v_sb.rearrange("p n (h x) -> p n h x", x=HD + 1)
            nc.vector.memset(v4[:, :, :, HD], 1.0)

            # ---- phase 1: qkv projection + rope + v pack ----
            with (
                tc.tile_pool(name="p1ps", bufs=2, space="PSUM") as pp,
                tc.tile_pool(name="p1t", bufs=3) as tp,
            ):
                for s in range(NS):
                    pq = pp.tile([P, NHC * HD], F32, tag="pq")
                    pk = pp.tile([P, NHC * HD], F32, tag="pk")
                    pv = pp.tile([P, NHC * HD], F32, tag="pv")
                    xsl = xt_sb[:, :, s * P : (s + 1) * P]
                    for c in range(NCC):
                        st, sp = (c == 0), (c == NCC - 1)
                        nc.tensor.matmul(
                            pq[:], xsl[:, c, :], wqkv_sb[:, c, 0:192],
                            start=st, stop=sp,
                        )
                        nc.tensor.matmul(
                            pk[:], xsl[:, c, :], wqkv_sb[:, c, 192:384],
                            start=st, stop=sp,
                        )
                        nc.tensor.matmul(
                            pv[:], xsl[:, c, :], wqkv_sb[:, c, 384:576],
                            start=st, stop=sp,
                        )
                    c3 = cos_sb[:, s, :].rearrange("p (b x) -> p b x", x=32)
                    s3 = sin_sb[:, s, :].rearrange("p (b x) -> p b x", x=32)
                    for src, dst in ((pq, q_sb), (pk, k_sb)):
                        sr3 = src.rearrange("p (b x) -> p b x", x=32)
                        t = tp.tile([P, NHC * HD], F32, tag="ropet")
                        t3 = t.rearrange("p (b x) -> p b x", x=32)
                        # t = shuffle(src) * sin_signed
                        nc.vector.tensor_tensor(
                            t3[:, :, 0:16], sr3[:, :, 16:32], s3[:, :, 0:16], ALU.mult
                        )
                        nc.vector.tensor_tensor(
                            t3[:, :, 16:32], sr3[:, :, 0:16], s3[:, :, 16:32], ALU.mult
                        )
                        t2 = tp.tile([P, NHC * HD], F32, tag="ropet2")
                        nc.vector.tensor_tensor(t2[:], src[:], cos_sb[:, s, :], ALU.mult)
                        nc.vector.tensor_tensor(dst[:, s, :], t2[:], t[:], ALU.add)
                    nc.scalar.copy(
                        v4[:, s, :, 0:HD],
                        pv.rearrange("p (h x) -> p h x", x=HD),
                    )

            # ---- phase 2: transpose q, k to d-major ----
            with tc.tile_pool(name="p2ps", bufs=4, space="PSUM") as p2:
                for s in range(NS):
                    sl = slice(s * P, (s + 1) * P)
                    for src, d01, d2 in ((q_sb, qt01, qt2), (k_sb, kt01, kt2)):
                        pt = p2.tile([P, P], BF, tag="pt")
                        nc.tensor.transpose(pt[:], src[:, s, 0:P], id_sb[:])
                        nc.scalar.copy(d01[:, sl], pt[:])
                        pt2 = p2.tile([P, P], BF, tag="pt")
                        nc.tensor.transpose(
                            pt2[0:HD, :], src[:, s, P : P + HD], id_sb[:]
                        )
                        nc.scalar.copy(d2[:, sl], pt2[0:HD, :])

            # ---- phase 3: attention (transposed scores) ----
            with (
                tc.tile_pool(name="scps", bufs=3, space="PSUM") as scp,
                tc.tile_pool(name="aops", bufs=2, space="PSUM") as aop,
                tc.tile_pool(name="denps", bufs=1, space="PSUM") as dnp,
                tc.tile_pool(name="expp", bufs=6) as expp,
            ):
                den_ps = dnp.tile([P, NHC, NS, 2], BF)  # pad: psum needs 4B align
                for h in range(NHC):
                    if h < 2:
                        kth, qth = kt01[h * HD : (h + 1) * HD], qt01[h * HD : (h + 1) * HD]
                    else:
                        kth, qth = kt2, qt2
                    for qj in range(NQ):
                        qsl = slice(qj * SQT, (qj + 1) * SQT)
                        pa = aop.tile([HD + 1, SQT], F32, tag="pa")
                        nki = KPQ * qj + KPQ
                        for ki in range(nki):
                            ps = scp.tile([P, SQT], F32, tag="ps")
                            nc.tensor.matmul(
                                ps[:],
                                kth[:, ki * P : (ki + 1) * P],
                                qth[:, qsl],
                                start=True, stop=True,
                            )
                            e = expp.tile([P, SQT], BF, tag="e")
                            nc.scalar.activation(e[:], ps[:], AF.Exp, scale=0.125)
                            r = ki - KPQ * qj
                            if r >= 0:
                                nc.gpsimd.tensor_tensor(
                                    e[:], e[:], mask_sb[:, r, :], ALU.mult
                                )
                            nc.tensor.matmul(
                                pa[:],
                                v_sb[:, ki, h * (HD + 1) : (h + 1) * (HD + 1)],
                                e[:],
                                start=(ki == 0), stop=(ki == nki - 1),
                            )
                        nc.scalar.copy(ao[h][:, qsl], pa[:])
                        # denominator row -> s-major columns (tiny PE transposes)
                        for c4 in range(KPQ):
                            col = qj * KPQ + c4
                            nc.tensor.transpose(
                                den_ps[:, h, col : col + 1, 0],
                                ao[h][HD : HD + 1, col * P : (col + 1) * P],
                                id_sb[HD : HD + 1, HD : HD + 1],
                            )
                nc.vector.reciprocal(den_rec[:], den_ps[:, :, :, 0])

            # ---- phase 4: per-head output projection + normalize ----
            with (
                tc.tile_pool(name="p4ps", bufs=2, space="PSUM") as p4,
                tc.tile_pool(name="outp", bufs=3) as op,
            ):
                wof = [wo_sb[:, h, :] for h in range(NHC)]
                for s in range(NS):
                    sl = slice(s * P, (s + 1) * P)
                    acc = op.tile([P, DM], F32, tag="acc")
                    for h in range(NHC):
                        po = p4.tile([P, DM], F32, tag="po")
                        lh = ao[h][0:HD, sl]
                        nc.tensor.matmul(
                            po[:, 0:512], lh, wof[h][:, 0:512], start=True, stop=True
                        )
                        nc.tensor.matmul(
                            po[:, 512:DM], lh, wof[h][:, 512:DM], start=True, stop=True
                        )
                        scale = den_rec[:, h, s : s + 1]
                        if h == 0:
                            nc.scalar.activation(acc[:], po[:], AF.Copy, scale=scale)
                        else:
                            nc.vector.scalar_tensor_tensor(
                                acc[:], po[:], scale, acc[:], ALU.mult, ALU.add
                            )
                    nc.sync.dma_start(out_d[sl, :], acc[:])

    nc.compile()
    return nc


_cache = {}
LAST_RESULT = None


def _get_program(S, n_devices):
    key = (S, n_devices)
    if key not in _cache:
        _cache[key] = build_program(S, n_devices)
    return _cache[key]


def _rope_tables(row_ids, col_ids, S):
    inv = 1.0 / (10000.0 ** (np.arange(0, 32, 2, dtype=np.float64) / 32.0))

    def block(ids):
        ang = ids.astype(np.float64)[:, None] * inv[None, :]
        c = np.concatenate([np.cos(ang), np.cos(ang)], -1)
        s_ = np.concatenate([-np.sin(ang), np.sin(ang)], -1)  # signed (shuffle form)
        return c, s_

    cr, sr = block(np.asarray(row_ids))
    cc, sc = block(np.asarray(col_ids))
    cos64 = np.concatenate([cr, cc], -1)
    sin64 = np.concatenate([sr, sc], -1)
    return (
        np.tile(cos64, (1, NHC)).astype(bfloat16),
        np.tile(sin64, (1, NHC)).astype(bfloat16),
    )


def _make_masks():
    pp_ = np.arange(P)[:, None]
    ff = np.arange(SQT)[None, :]
    m = np.zeros((P, SQT // P, SQT), np.float32)
    for r in range(SQT // P):
        m[:, r, :] = (pp_ <= ff - P * r).astype(np.float32)
    return m.astype(bfloat16)


def kernel(x, row_ids, col_ids, Wq, Wk, Wv, Wo):
    x = np.asarray(x)
    B, S, _ = x.shape
    n_cores = 8
    groups = n_cores // B  # head groups per batch (4)
    hpg = NHC  # heads per group

    nc = _get_program(S, n_cores)
    cos_t, sin_t = _rope_tables(row_ids, col_ids, S)
    masks = _make_masks()
    ident = np.eye(P, dtype=bfloat16)

    Wq, Wk, Wv, Wo = (np.asarray(w, np.float32) for w in (Wq, Wk, Wv, Wo))
    in_maps = []
    for c in range(n_cores):
        b = c // groups
        h0 = (c % groups) * hpg
        rows = slice(h0 * HD, (h0 + hpg) * HD)
        xt = np.ascontiguousarray(x[b].T).astype(bfloat16)
        wqkv = np.concatenate(
            [Wq[rows].T, Wk[rows].T, Wv[rows].T], axis=1
        ).astype(bfloat16)
        wo = np.ascontiguousarray(Wo[:, rows].T).astype(bfloat16)
        in_maps.append(
            {
                "xt": xt,
                "wqkv": wqkv,
                "wo": wo,
                "cos": cos_t,
                "sin": sin_t,
                "masks": masks,
                "ident": ident,
            }
        )

    import os

    trace = bool(os.environ.get("KERNEL_TRACE"))
    kw = {}
    if trace and os.environ.get("KERNEL_TRACE_DIR"):
        kw["tmpdir"] = os.environ["KERNEL_TRACE_DIR"]
    res = run_bass_kernel_spmd(nc, in_maps, list(range(n_cores)), trace=trace, **kw)
    global LAST_RESULT
    LAST_RESULT = res
    outs = [res.results[c]["outp"] for c in range(n_cores)]
    out = np.stack(
        [sum(outs[b * groups + g] for g in range(groups)) for b in range(B)], axis=0
    )
    return out.astype(np.float32)


# revision 15
# speedup vs baseline: 1.0319x; 1.0319x over previous
"""Trainium2 Bass kernel for causal self-attention with 2D RoPE.

Sharding: batch x head-group parallel over 8 NeuronCores.
  core c -> batch b = c // 4, heads h0 = (c % 4) * 3 .. h0+2.
Each core computes q/k/v projections for its 3 heads, 2D RoPE, causal
flash-attention (transposed-score layout, denominator via an appended
ones-column on V), and a per-head output projection with the softmax
normalization folded into the PSUM eviction scale. The host sums the
4 partial outputs per batch.

Matmuls run in bf16 (fp32 PSUM accumulation).
"""

import sys

sys.path.insert(0, "/opt/trn_rl_repo")

import numpy as np
from ml_dtypes import bfloat16

import concourse.bacc as bacc
import concourse.mybir as mybir
from concourse import tile
from concourse.bass_utils import run_bass_kernel_spmd

BF = mybir.dt.bfloat16
F32 = mybir.dt.float32
AF = mybir.ActivationFunctionType
ALU = mybir.AluOpType

P = 128          # partitions
DM = 768         # d_model
HD = 64          # head dim
NHC = 3          # heads per core
NCC = DM // P    # contraction chunks (6)
SQT = 512        # q-block (matmul moving dim)
QKV = 3 * NHC * HD  # 576
VW = NHC * (HD + 1)  # v + ones columns (195)


def build_program(S=2048, n_devices=8):
    NS = S // P      # seq chunks of 128
    NQ = S // SQT    # q blocks of 512
    KPQ = SQT // P   # k-chunks per q-block (4)

    nc = bacc.Bacc(
        "TRN2", target_bir_lowering=False, debug=False, num_devices=n_devices
    )
    xt_d = nc.dram_tensor("xt", [DM, S], BF, kind="ExternalInput")
    wqkv_d = nc.dram_tensor("wqkv", [DM, QKV], BF, kind="ExternalInput")
    wo_d = nc.dram_tensor("wo", [NHC * HD, DM], BF, kind="ExternalInput")
    cos_d = nc.dram_tensor("cos", [S, 2 * NHC * HD], BF, kind="ExternalInput")
    sin_d = nc.dram_tensor("sin", [S, 2 * NHC * HD], BF, kind="ExternalInput")
    mask_d = nc.dram_tensor("masks", [P, P], BF, kind="ExternalInput")
    id_d = nc.dram_tensor("ident", [P, P], BF, kind="ExternalInput")
    out_d = nc.dram_tensor("outp", [S, DM], F32, kind="ExternalOutput")

    with tile.TileContext(nc) as tc:
        with (
            tc.tile_pool(name="const", bufs=1) as const,
            tc.tile_pool(name="resid", bufs=1) as resid,
        ):
            xt_sb = const.tile([P, NCC, S], BF)
            for c in range(NCC):
                nc.sync.dma_start(xt_sb[:, c, :], xt_d[c * P : (c + 1) * P, :])
            wqkv_sb = const.tile([P, NCC, QKV], BF)
            nc.sync.dma_start(wqkv_sb[:], wqkv_d.rearrange("(n p) m -> p n m", p=P))
            wo_sb = const.tile([HD, NHC, DM], BF)
            for h in range(NHC):
                nc.sync.dma_start(wo_sb[:, h, :], wo_d[h * HD : (h + 1) * HD, :])
            cos_sb = const.tile([P, NS, 2 * NHC * HD], BF)
            nc.sync.dma_start(cos_sb[:], cos_d.rearrange("(n p) m -> p n m", p=P))
            sin_sb = const.tile([P, NS, 2 * NHC * HD], BF)
            nc.sync.dma_start(sin_sb[:], sin_d.rearrange("(n p) m -> p n m", p=P))
            mask_sb = const.tile([P, P], BF)  # diag: 1 if p <= f else 0
            nc.sync.dma_start(mask_sb[:], mask_d[:])
            id_sb = const.tile([P, P], BF)
            nc.sync.dma_start(id_sb[:], id_d[:])

            q_sb = resid.tile([P, NS, NHC * HD], BF)
            k_sb = resid.tile([P, NS, NHC * HD], BF)
            v_sb = resid.tile([P, NS, VW], BF)
            qt01 = resid.tile([P, S], BF)
            qt2 = resid.tile([HD, S], BF)
            kt01 = resid.tile([P, S], BF)
            kt2 = resid.tile([HD, S], BF)
            ao = [resid.tile([HD + 1, S], BF, name=f"ao{h}") for h in range(NHC)]
            den_rec = resid.tile([P, NHC, NS], F32)

            # ones columns of v (denominator trick)
            v4 = v_sb.rearrange("p n (h x) -> p n h x", x=HD + 1)
            nc.vector.memset(v4[:, :, :, HD], 1.0)

            # ---- phase 1: qkv projection + rope + v pack ----
            with (
                tc.tile_pool(name="p1ps", bufs=2, space="PSUM") as pp,
                tc.tile_pool(name="p1t", bufs=3) as tp,
            ):
                for s in range(NS):
                    pqkv = pp.tile([P, QKV], F32, tag="pqkv")
                    xsl = xt_sb[:, :, s * P : (s + 1) * P]
                    for c in range(NCC):
                        st, sp = (c == 0), (c == NCC - 1)
                        nc.tensor.matmul(
                            pqkv[:, 0:512], xsl[:, c, :], wqkv_sb[:, c, 0:512],
                            start=st, stop=sp,
                        )
                        nc.tensor.matmul(
                            pqkv[:, 512:QKV], xsl[:, c, :], wqkv_sb[:, c, 512:QKV],
                            start=st, stop=sp,
                        )
                    # rope on q and k together ([:, 0:384] of the psum tile)
                    qk = pqkv[:, 0:384]
                    qk3 = qk.rearrange("p (b x) -> p b x", x=32)
                    c3 = cos_sb[:, s, :]
                    s3 = sin_sb[:, s, :].rearrange("p (b x) -> p b x", x=32)
                    t = tp.tile([P, 384], F32, tag="ropet")
                    t3 = t.rearrange("p (b x) -> p b x", x=32)
                    # t = shuffle(qk) * sin_signed  (swap 16-halves per 32-block)
                    nc.vector.tensor_tensor(
                        t3[:, :, 0:16], qk3[:, :, 16:32], s3[:, :, 0:16], ALU.mult
                    )
                    nc.vector.tensor_tensor(
                        t3[:, :, 16:32], qk3[:, :, 0:16], s3[:, :, 16:32], ALU.mult
                    )
                    t2 = tp.tile([P, 384], F32, tag="ropet2")
                    nc.vector.tensor_tensor(t2[:], qk[:], c3[:, 0:384], ALU.mult)
                    # final add on gpsimd (sbuf-only engine), cast to bf16
                    nc.gpsimd.tensor_tensor(q_sb[:, s, :], t2[:, 0:192], t[:, 0:192], ALU.add)
                    nc.gpsimd.tensor_tensor(k_sb[:, s, :], t2[:, 192:384], t[:, 192:384], ALU.add)
                    # v pack with ones columns
                    nc.scalar.copy(
                        v4[:, s, :, 0:HD],
                        pqkv[:, 384:QKV].rearrange("p (h x) -> p h x", x=HD),
                    )

            # ---- phase 2: transpose q, k to d-major ----
            with tc.tile_pool(name="p2ps", bufs=4, space="PSUM") as p2:
                for s in range(NS):
                    sl = slice(s * P, (s + 1) * P)
                    for src, d01, d2, ev in (
                        (k_sb, kt01, kt2, "v"),
                        (q_sb, qt01, qt2, "a"),
                    ):
                        pt = p2.tile([P, P], BF, tag="pt")
                        nc.tensor.transpose(pt[:], src[:, s, 0:P], id_sb[:])
                        pt2 = p2.tile([P, P], BF, tag="pt")
                        nc.tensor.transpose(
                            pt2[0:HD, :], src[:, s, P : P + HD], id_sb[:]
                        )
                        if ev == "a":
                            nc.scalar.copy(d01[:, sl], pt[:])
                            nc.scalar.copy(d2[:, sl], pt2[0:HD, :])
                        else:
                            nc.vector.tensor_copy(d01[:, sl], pt[:])
                            nc.vector.tensor_copy(d2[:, sl], pt2[0:HD, :])

            # ---- phase 3: attention (transposed scores) ----
            with (
                tc.tile_pool(name="scps", bufs=4, space="PSUM") as scp,
                tc.tile_pool(name="aops", bufs=3, space="PSUM") as aop,
                tc.tile_pool(name="denps", bufs=1, space="PSUM") as dnp,
                tc.tile_pool(name="expp", bufs=8) as expp,
            ):
                den_ps = dnp.tile([P, NHC, NS, 2], BF)  # pad: psum needs 4B align
                for h in range(NHC):
                    if h < 2:
                        kth = kt01[h * HD : (h + 1) * HD]
                        qth = qt01[h * HD : (h + 1) * HD]
                    else:
                        kth, qth = kt2, qt2
                    for qj in range(NQ):
                        pa = aop.tile([HD + 1, SQT], F32, tag="pa")
                        nki = KPQ * qj + KPQ
                        for ki in range(nki):
                            r = ki - KPQ * qj  # >= 0: diagonal-crossing tile
                            off = max(r, 0) * P
                            ps = scp.tile([P, SQT], F32, tag="ps")
                            nc.tensor.matmul(
                                ps[:, off:SQT],
                                kth[:, ki * P : (ki + 1) * P],
                                qth[:, qj * SQT + off : (qj + 1) * SQT],
                                start=True, stop=True,
                            )
                            e = expp.tile([P, SQT], BF, tag="e")
                            nc.scalar.activation(
                                e[:, off:SQT], ps[:, off:SQT], AF.Exp, scale=0.125
                            )
                            if r >= 0:
                                if off > 0:
                                    nc.vector.memset(e[:, 0:off], 0.0)
                                nc.vector.tensor_tensor(
                                    e[:, off : off + P],
                                    e[:, off : off + P],
                                    mask_sb[:],
                                    ALU.mult,
                                )
                            nc.tensor.matmul(
                                pa[:],
                                v_sb[:, ki, h * (HD + 1) : (h + 1) * (HD + 1)],
                                e[:],
                                start=(ki == 0), stop=(ki == nki - 1),
                            )
                        nc.scalar.copy(ao[h][:, qj * SQT : (qj + 1) * SQT], pa[:])
                        # denominator row -> s-major columns (tiny PE transposes)
                        for c4 in range(KPQ):
                            col = qj * KPQ + c4
                            nc.tensor.transpose(
                                den_ps[:, h, col : col + 1, 0],
                                ao[h][HD : HD + 1, col * P : (col + 1) * P],
                                id_sb[HD : HD + 1, HD : HD + 1],
                            )
                nc.vector.reciprocal(den_rec[:], den_ps[:, :, :, 0])

            # ---- phase 4: per-head output projection + normalize ----
            with (
                tc.tile_pool(name="p4ps", bufs=2, space="PSUM") as p4,
                tc.tile_pool(name="outp", bufs=3) as op,
            ):
                wof = [wo_sb[:, h, :] for h in range(NHC)]
                for s in range(NS):
                    sl = slice(s * P, (s + 1) * P)
                    acc = op.tile([P, DM], F32, tag="acc")
                    for h in range(NHC):
                        po = p4.tile([P, DM], F32, tag="po")
                        lh = ao[h][0:HD, sl]
                        nc.tensor.matmul(
                            po[:, 0:512], lh, wof[h][:, 0:512], start=True, stop=True
                        )
                        nc.tensor.matmul(
                            po[:, 512:DM], lh, wof[h][:, 512:DM], start=True, stop=True
                        )
                        scale = den_rec[:, h, s : s + 1]
                        if h == 0:
                            nc.scalar.activation(acc[:], po[:], AF.Copy, scale=scale)
                        else:
                            nc.vector.scalar_tensor_tensor(
                                acc[:], po[:], scale, acc[:], ALU.mult, ALU.add
                            )
                    nc.sync.dma_start(out_d[sl, :], acc[:])

    nc.compile()
    return nc


_cache = {}
LAST_RESULT = None


def _get_program(S, n_devices):
    key = (S, n_devices)
    if key not in _cache:
        _cache[key] = build_program(S, n_devices)
    return _cache[key]


def _rope_tables(row_ids, col_ids, S):
    inv = 1.0 / (10000.0 ** (np.arange(0, 32, 2, dtype=np.float64) / 32.0))

    def block(ids):
        ang = ids.astype(np.float64)[:, None] * inv[None, :]
        c = np.concatenate([np.cos(ang), np.cos(ang)], -1)
        s_ = np.concatenate([-np.sin(ang), np.sin(ang)], -1)  # signed (shuffle form)
        return c, s_

    cr, sr = block(np.asarray(row_ids))
    cc, sc = block(np.asarray(col_ids))
    cos64 = np.concatenate([cr, cc], -1)
    sin64 = np.concatenate([sr, sc], -1)
    return (
        np.tile(cos64, (1, 2 * NHC)).astype(bfloat16),
        np.tile(sin64, (1, 2 * NHC)).astype(bfloat16),
    )


def _make_masks():
    pp_ = np.arange(P)[:, None]
    ff = np.arange(P)[None, :]
    return (pp_ <= ff).astype(np.float32).astype(bfloat16)


def kernel(x, row_ids, col_ids, Wq, Wk, Wv, Wo):
    x = np.asarray(x)
    B, S, _ = x.shape
    n_cores = 8
    groups = n_cores // B  # head groups per batch (4)
    hpg = NHC  # heads per group

    nc = _get_program(S, n_cores)
    cos_t, sin_t = _rope_tables(row_ids, col_ids, S)
    masks = _make_masks()
    ident = np.eye(P, dtype=bfloat16)

    Wq, Wk, Wv, Wo = (np.asarray(w, np.float32) for w in (Wq, Wk, Wv, Wo))
    in_maps = []
    for c in range(n_cores):
        b = c // groups
        h0 = (c % groups) * hpg
        rows = slice(h0 * HD, (h0 + hpg) * HD)
        xt = np.ascontiguousarray(x[b].T).astype(bfloat16)
        wqkv = np.concatenate(
            [Wq[rows].T, Wk[rows].T, Wv[rows].T], axis=1
        ).astype(bfloat16)
        wo = np.ascontiguousarray(Wo[:, rows].T).astype(bfloat16)
        in_maps.append(
            {
                "xt": xt,
                "wqkv": wqkv,
                "wo": wo,
                "cos": cos_t,
                "sin": sin_t,
                "masks": masks,
                "ident": ident,
            }
        )

    import os

    trace = bool(os.environ.get("KERNEL_TRACE"))
    kw = {}
    if trace and os.environ.get("KERNEL_TRACE_DIR"):
        kw["tmpdir"] = os.environ["KERNEL_TRACE_DIR"]
    res = run_bass_kernel_spmd(nc, in_maps, list(range(n_cores)), trace=trace, **kw)
    global LAST_RESULT
    LAST_RESULT = res

    outs = [res.results[c]["outp"] for c in range(n_cores)]
    out = np.stack(
        [sum(outs[b * groups + g] for g in range(groups)) for b in range(B)], axis=0
    )
    return out.astype(np.float32)


# revision 24
# speedup vs baseline: 1.0857x; 1.0521x over previous
"""Trainium2 Bass kernel for causal self-attention with 2D RoPE.

Sharding: batch x head-group parallel over 8 NeuronCores.
  core c -> batch b = c // 4, heads h0 = (c % 4) * 3 .. h0+2.
Each core computes q/k/v projections for its 3 heads, 2D RoPE, causal
flash-attention (transposed-score layout, denominator via an appended
ones-column on V), and a per-head output projection with the softmax
normalization folded into the PSUM eviction scale. The host sums the
4 partial outputs per batch.

Matmuls run in bf16 (fp32 PSUM accumulation).
"""

import sys

sys.path.insert(0, "/opt/trn_rl_repo")

import numpy as np
from ml_dtypes import bfloat16

import concourse.bacc as bacc
import concourse.mybir as mybir
from concourse import tile
from concourse.bass_utils import run_bass_kernel_spmd

BF = mybir.dt.bfloat16
F32 = mybir.dt.float32
AF = mybir.ActivationFunctionType
ALU = mybir.AluOpType

P = 128          # partitions
DM = 768         # d_model
HD = 64          # head dim
NHC = 3          # heads per core
NCC = DM // P    # contraction chunks (6)
SQT = 512        # q-block (matmul moving dim)
QKV = 3 * NHC * HD  # 576
VW = NHC * (HD + 1)  # v + ones columns (195)


def build_program(S=2048, n_devices=8):
    NS = S // P      # seq chunks of 128
    NQ = S // SQT    # q blocks of 512
    KPQ = SQT // P   # k-chunks per q-block (4)

    nc = bacc.Bacc(
        "TRN2", target_bir_lowering=False, debug=False, num_devices=n_devices
    )
    xt_d = nc.dram_tensor("xt", [DM, S], BF, kind="ExternalInput")
    wqkv_d = nc.dram_tensor("wqkv", [DM, QKV], BF, kind="ExternalInput")
    wo_d = nc.dram_tensor("wo", [NHC * HD, DM], BF, kind="ExternalInput")
    cos_d = nc.dram_tensor("cos", [S, 2 * NHC * HD], BF, kind="ExternalInput")
    sin_d = nc.dram_tensor("sin", [S, 2 * NHC * HD], BF, kind="ExternalInput")
    mask_d = nc.dram_tensor("masks", [P, P], BF, kind="ExternalInput")
    id_d = nc.dram_tensor("ident", [P, P], BF, kind="ExternalInput")
    out_d = nc.dram_tensor("outp", [S, DM], F32, kind="ExternalOutput")

    with tile.TileContext(nc) as tc:
        with (
            tc.tile_pool(name="const", bufs=1) as const,
            tc.tile_pool(name="resid", bufs=1) as resid,
        ):
            wqkv_sb = const.tile([P, NCC, QKV], BF)
            nc.sync.dma_start(wqkv_sb[:], wqkv_d.rearrange("(n p) m -> p n m", p=P))
            xt_sb = const.tile([P, NCC, S], BF)
            for sb in range(S // SQT):
                ssl = slice(sb * SQT, (sb + 1) * SQT)
                for c in range(NCC):
                    nc.sync.dma_start(
                        xt_sb[:, c, ssl], xt_d[c * P : (c + 1) * P, ssl]
                    )
            wo_sb = const.tile([HD, NHC, DM], BF)
            for h in range(NHC):
                nc.sync.dma_start(wo_sb[:, h, :], wo_d[h * HD : (h + 1) * HD, :])
            cos_sb = const.tile([P, NS, 2 * NHC * HD], BF)
            nc.sync.dma_start(cos_sb[:], cos_d.rearrange("(n p) m -> p n m", p=P))
            sin_sb = const.tile([P, NS, 2 * NHC * HD], BF)
            nc.sync.dma_start(sin_sb[:], sin_d.rearrange("(n p) m -> p n m", p=P))
            mask_sb = const.tile([P, P], BF)  # diag: 1 if p <= f else 0
            nc.sync.dma_start(mask_sb[:], mask_d[:])
            id_sb = const.tile([P, P], BF)
            nc.sync.dma_start(id_sb[:], id_d[:])

            q_sb = resid.tile([P, NS, NHC * HD], BF)
            k_sb = resid.tile([P, NS, NHC * HD], BF)
            v_sb = resid.tile([P, NS, VW], BF)
            qt01 = resid.tile([P, S], BF)
            qt2 = resid.tile([HD, S], BF)
            kt01 = resid.tile([P, S], BF)
            kt2 = resid.tile([HD, S], BF)
            ao = [resid.tile([HD + 1, S], BF, name=f"ao{h}") for h in range(NHC)]
            den_rec = resid.tile([P, NHC, NS], F32)

            # ones columns of v (denominator trick)
            v4 = v_sb.rearrange("p n (h x) -> p n h x", x=HD + 1)
            nc.vector.memset(v4[:, :, :, HD], 1.0)

            # ---- phase 1: qkv projection + rope + v pack ----
            with (
                tc.tile_pool(name="p1ps", bufs=3, space="PSUM") as pp,
                tc.tile_pool(name="p1t", bufs=3) as tp,
            ):
                for s in range(NS):
                    pqkv = pp.tile([P, QKV], F32, tag="pqkv")
                    xsl = xt_sb[:, :, s * P : (s + 1) * P]
                    for c in range(NCC):
                        st, sp = (c == 0), (c == NCC - 1)
                        nc.tensor.matmul(
                            pqkv[:, 0:512], xsl[:, c, :], wqkv_sb[:, c, 0:512],
                            start=st, stop=sp,
                        )
                        nc.tensor.matmul(
                            pqkv[:, 512:QKV], xsl[:, c, :], wqkv_sb[:, c, 512:QKV],
                            start=st, stop=sp,
                        )
                    # rope on q and k together ([:, 0:384] of the psum tile)
                    qk = pqkv[:, 0:384]
                    qk3 = qk.rearrange("p (b x) -> p b x", x=32)
                    c3 = cos_sb[:, s, :]
                    s3 = sin_sb[:, s, :].rearrange("p (b x) -> p b x", x=32)
                    t = tp.tile([P, 384], F32, tag="ropet")
                    t3 = t.rearrange("p (b x) -> p b x", x=32)
                    # t = shuffle(qk) * sin_signed  (swap 16-halves per 32-block)
                    nc.vector.tensor_tensor(
                        t3[:, :, 0:16], qk3[:, :, 16:32], s3[:, :, 0:16], ALU.mult
                    )
                    nc.vector.tensor_tensor(
                        t3[:, :, 16:32], qk3[:, :, 0:16], s3[:, :, 16:32], ALU.mult
                    )
                    t2 = tp.tile([P, 384], F32, tag="ropet2")
                    nc.vector.tensor_tensor(t2[:], qk[:], c3[:, 0:384], ALU.mult)
                    # final add on gpsimd (sbuf-only engine), cast to bf16
                    nc.gpsimd.tensor_tensor(q_sb[:, s, :], t2[:, 0:192], t[:, 0:192], ALU.add)
                    nc.gpsimd.tensor_tensor(k_sb[:, s, :], t2[:, 192:384], t[:, 192:384], ALU.add)
                    # v pack with ones columns
                    nc.scalar.copy(
                        v4[:, s, :, 0:HD],
                        pqkv[:, 384:QKV].rearrange("p (h x) -> p h x", x=HD),
                    )

            # ---- phase 2: transpose q, k to d-major (PE transposes) ----
            with tc.tile_pool(name="p2ps", bufs=4, space="PSUM") as p2:
                for s in range(NS):
                    sl = slice(s * P, (s + 1) * P)
                    for src_t, d01, d2, ev in (
                        (k_sb, kt01, kt2, "v"),
                        (q_sb, qt01, qt2, "a"),
                    ):
                        pt = p2.tile([P, P], BF, tag="pt")
                        nc.tensor.transpose(pt[:], src_t[:, s, 0:P], id_sb[:])
                        pt2 = p2.tile([P, P], BF, tag="pt")
                        nc.tensor.transpose(
                            pt2[0:HD, :], src_t[:, s, P : P + HD], id_sb[:]
                        )
                        if ev == "a":
                            nc.scalar.copy(d01[:, sl], pt[:])
                            nc.scalar.copy(d2[:, sl], pt2[0:HD, :])
                        else:
                            nc.vector.tensor_copy(d01[:, sl], pt[:])
                            nc.vector.tensor_copy(d2[:, sl], pt2[0:HD, :])

            # ---- phase 3: attention (transposed scores) ----
            with (
                tc.tile_pool(name="scps", bufs=5, space="PSUM") as scp,
                tc.tile_pool(name="aops", bufs=2, space="PSUM") as aop,
                tc.tile_pool(name="denps", bufs=1, space="PSUM") as dnp,
                tc.tile_pool(name="expp", bufs=8) as expp,
            ):
                den_ps = dnp.tile([P, NHC, NS, 2], BF)  # pad: psum needs 4B align
                for h in range(NHC):
                    if h < 2:
                        kth = kt01[h * HD : (h + 1) * HD]
                        qth = qt01[h * HD : (h + 1) * HD]
                    else:
                        kth, qth = kt2, qt2
                    for qj in range(NQ):
                        pa = aop.tile([HD + 1, SQT], F32, tag="pa")
                        nki = KPQ * qj + KPQ
                        for ki in range(nki):
                            r = ki - KPQ * qj  # >= 0: diagonal-crossing tile
                            off = max(r, 0) * P
                            ps = scp.tile([P, SQT], F32, tag="ps")
                            nc.tensor.matmul(
                                ps[:, off:SQT],
                                kth[:, ki * P : (ki + 1) * P],
                                qth[:, qj * SQT + off : (qj + 1) * SQT],
                                start=True, stop=True,
                            )
                            e = expp.tile([P, SQT], BF, tag="e")
                            nc.scalar.activation(
                                e[:, off:SQT], ps[:, off:SQT], AF.Exp, scale=0.125
                            )
                            if r >= 0:
                                if off > 0:
                                    nc.vector.memset(e[:, 0:off], 0.0)
                                nc.vector.tensor_tensor(
                                    e[:, off : off + P],
                                    e[:, off : off + P],
                                    mask_sb[:],
                                    ALU.mult,
                                )
                            nc.tensor.matmul(
                                pa[:],
                                v_sb[:, ki, h * (HD + 1) : (h + 1) * (HD + 1)],
                                e[:],
                                start=(ki == 0), stop=(ki == nki - 1),
                            )
                        nc.scalar.copy(ao[h][:, qj * SQT : (qj + 1) * SQT], pa[:])
                        # denominator row -> s-major columns (tiny PE transposes)
                        for c4 in range(KPQ):
                            col = qj * KPQ + c4
                            nc.tensor.transpose(
                                den_ps[:, h, col : col + 1, 0],
                                ao[h][HD : HD + 1, col * P : (col + 1) * P],
                                id_sb[HD : HD + 1, HD : HD + 1],
                            )
                nc.vector.reciprocal(den_rec[:], den_ps[:, :, :, 0])

            # ---- phase 4: per-head output projection + normalize ----
            with (
                tc.tile_pool(name="p4ps", bufs=3, space="PSUM") as p4,
                tc.tile_pool(name="outp", bufs=3) as op,
            ):
                wof = [wo_sb[:, h, :] for h in range(NHC)]
                for s in range(NS):
                    sl = slice(s * P, (s + 1) * P)
                    acc = op.tile([P, DM], F32, tag="acc")
                    for h in range(NHC):
                        po = p4.tile([P, DM], F32, tag="po")
                        lh = ao[h][0:HD, sl]
                        nc.tensor.matmul(
                            po[:, 0:512], lh, wof[h][:, 0:512], start=True, stop=True
                        )
                        nc.tensor.matmul(
                            po[:, 512:DM], lh, wof[h][:, 512:DM], start=True, stop=True
                        )
                        scale = den_rec[:, h, s : s + 1]
                        if h == 0:
                            nc.scalar.activation(acc[:], po[:], AF.Copy, scale=scale)
                        else:
                            nc.vector.scalar_tensor_tensor(
                                acc[:], po[:], scale, acc[:], ALU.mult, ALU.add
                            )
                    nc.sync.dma_start(out_d[sl, :], acc[:])

    nc.compile()
    return nc


_cache = {}
LAST_RESULT = None


def _get_program(S, n_devices):
    key = (S, n_devices)
    if key not in _cache:
        _cache[key] = build_program(S, n_devices)
    return _cache[key]


def _rope_tables(row_ids, col_ids, S):
    inv = 1.0 / (10000.0 ** (np.arange(0, 32, 2, dtype=np.float64) / 32.0))

    def block(ids):
        ang = ids.astype(np.float64)[:, None] * inv[None, :]
        c = np.concatenate([np.cos(ang), np.cos(ang)], -1)
        s_ = np.concatenate([-np.sin(ang), np.sin(ang)], -1)  # signed (shuffle form)
        return c, s_

    cr, sr = block(np.asarray(row_ids))
    cc, sc = block(np.asarray(col_ids))
    cos64 = np.concatenate([cr, cc], -1)
    sin64 = np.concatenate([sr, sc], -1)
    return (
        np.tile(cos64, (1, 2 * NHC)).astype(bfloat16),
        np.tile(sin64, (1, 2 * NHC)).astype(bfloat16),
    )


def _make_masks():
    pp_ = np.arange(P)[:, None]
    ff = np.arange(P)[None, :]
    return (pp_ <= ff).astype(np.float32).astype(bfloat16)


def kernel(x, row_ids, col_ids, Wq, Wk, Wv, Wo):
    x = np.asarray(x)
    B, S, _ = x.shape
    n_cores = 8
    groups = n_cores // B  # head groups per batch (4)
    hpg = NHC  # heads per group

    nc = _get_program(S, n_cores)
    cos_t, sin_t = _rope_tables(row_ids, col_ids, S)
    masks = _make_masks()
    ident = np.eye(P, dtype=bfloat16)

    Wq, Wk, Wv, Wo = (np.asarray(w, np.float32) for w in (Wq, Wk, Wv, Wo))
    in_maps = []
    for c in range(n_cores):
        b = c // groups
        h0 = (c % groups) * hpg
        rows = slice(h0 * HD, (h0 + hpg) * HD)
        xt = np.ascontiguousarray(x[b].T).astype(bfloat16)
        wqkv = np.concatenate(
            [Wq[rows].T, Wk[rows].T, Wv[rows].T], axis=1
        ).astype(bfloat16)
        wo = np.ascontiguousarray(Wo[:, rows].T).astype(bfloat16)
        in_maps.append(
            {
                "xt": xt,
                "wqkv": wqkv,
                "wo": wo,
                "cos": cos_t,
                "sin": sin_t,
                "masks": masks,
                "ident": ident,
            }
        )

    import os

    trace = bool(os.environ.get("KERNEL_TRACE"))
    kw = {}
    if trace and os.environ.get("KERNEL_TRACE_DIR"):
        kw["tmpdir"] = os.environ["KERNEL_TRACE_DIR"]
    res = run_bass_kernel_spmd(nc, in_maps, list(range(n_cores)), trace=trace, **kw)
    global LAST_RESULT
    LAST_RESULT = res

    outs = [res.results[c]["outp"] for c in range(n_cores)]
    out = np.stack(
        [sum(outs[b * groups + g] for g in range(groups)) for b in range(B)], axis=0
    )
    return out.astype(np.float32)


# revision 27
# speedup vs baseline: 1.3741x; 1.2656x over previous
"""Trainium2 Bass kernel for causal self-attention with 2D RoPE.

Sharding: batch x head-group parallel over 8 NeuronCores.
  core c -> batch b = c // 4, heads h0 = (c % 4) * 3 .. h0+2.
Each core computes q/k/v projections for its 3 heads, 2D RoPE, causal
flash-attention (transposed-score layout, denominator via an appended
ones-column on V), and a per-head output projection with the softmax
normalization folded into the PSUM eviction scale. The host sums the
4 partial outputs per batch.

Matmuls run in bf16 (fp32 PSUM accumulation). All matmuls are zero-padded
to K=128 contraction so the PE activity monitor keeps the clock at 2.4GHz.
"""

import sys

sys.path.insert(0, "/opt/trn_rl_repo")

import numpy as np
from ml_dtypes import bfloat16

import concourse.bacc as bacc
import concourse.mybir as mybir
from concourse import tile
from concourse.bass_utils import run_bass_kernel_spmd

BF = mybir.dt.bfloat16
F32 = mybir.dt.float32
AF = mybir.ActivationFunctionType
ALU = mybir.AluOpType

P = 128          # partitions
DM = 768         # d_model
HD = 64          # head dim
NHC = 3          # heads per core
NCC = DM // P    # contraction chunks (6)
SQT = 512        # q-block (matmul moving dim)
QKV = 3 * NHC * HD  # 576


def build_program(S=2048, n_devices=8):
    NS = S // P      # seq chunks of 128
    NQ = S // SQT    # q blocks of 512
    KPQ = SQT // P   # k-chunks per q-block (4)

    nc = bacc.Bacc(
        "TRN2", target_bir_lowering=False, debug=False, num_devices=n_devices
    )
    xt_d = nc.dram_tensor("xt", [DM, S], BF, kind="ExternalInput")
    wqkv_d = nc.dram_tensor("wqkv", [DM, QKV], BF, kind="ExternalInput")
    wo_d = nc.dram_tensor("wo", [NHC * HD, DM], BF, kind="ExternalInput")
    cos_d = nc.dram_tensor("cos", [S, 2 * NHC * HD], BF, kind="ExternalInput")
    sin_d = nc.dram_tensor("sin", [S, 2 * NHC * HD], BF, kind="ExternalInput")
    mask_d = nc.dram_tensor("masks", [P, P], BF, kind="ExternalInput")
    id_d = nc.dram_tensor("ident", [P, P], BF, kind="ExternalInput")
    out_d = nc.dram_tensor("outp", [S, DM], F32, kind="ExternalOutput")

    with tile.TileContext(nc) as tc:
        with (
            tc.tile_pool(name="const", bufs=1) as const,
            tc.tile_pool(name="resid", bufs=1) as resid,
        ):
            # weights first (needed by the first matmul), xt in column blocks
            wqkv_sb = const.tile([P, NCC, QKV], BF)
            for c in range(NCC):
                nc.sync.dma_start(
                    wqkv_sb[:, c, :], wqkv_d[c * P : (c + 1) * P, :]
                )
            XBLK = 1024
            xt_sb = const.tile([P, NCC, S], BF)
            for sb in range(S // XBLK):
                ssl = slice(sb * XBLK, (sb + 1) * XBLK)
                for c in range(NCC):
                    eng = nc.sync if c % 2 == 0 else nc.scalar
                    eng.dma_start(xt_sb[:, c, ssl], xt_d[c * P : (c + 1) * P, ssl])
            wo_sb = const.tile([P, NHC, DM], BF)  # rows HD:P zeroed (K pad)
            for h in range(NHC):
                nc.scalar.dma_start(wo_sb[0:HD, h, :], wo_d[h * HD : (h + 1) * HD, :])
            cos_sb = const.tile([P, NS, 2 * NHC * HD], BF)
            nc.scalar.dma_start(cos_sb[:], cos_d.rearrange("(n p) m -> p n m", p=P))
            sin_sb = const.tile([P, NS, 2 * NHC * HD], BF)
            nc.scalar.dma_start(sin_sb[:], sin_d.rearrange("(n p) m -> p n m", p=P))
            mask_sb = const.tile([P, P], BF)  # diag: 1 if p <= f else 0
            nc.scalar.dma_start(mask_sb[:], mask_d[:])
            id_sb = const.tile([P, P], BF)
            nc.scalar.dma_start(id_sb[:], id_d[:])

            q_sb = resid.tile([P, NS, NHC * HD], BF)
            k_sb = resid.tile([P, NS, NHC * HD], BF)
            v_sb = resid.tile([P, NS, NHC, P], BF)  # per-head 128 cols (M pad)
            qtz = [resid.tile([P, S], BF, name=f"qtz{h}") for h in range(NHC)]
            ktz = [resid.tile([P, S], BF, name=f"ktz{h}") for h in range(NHC)]
            ao = [resid.tile([P, S], BF, name=f"ao{h}") for h in range(NHC)]
            den_rec = resid.tile([P, NHC, NS], F32)

            # one-time zero/one fills (gpsimd, overlaps the input DMAs)
            nc.gpsimd.memset(v_sb[:, :, :, HD], 1.0)       # denominator ones
            nc.gpsimd.memset(v_sb[:, :, :, HD + 1 : P], 0.0)  # M pad
            nc.gpsimd.memset(wo_sb[HD:P, :, :], 0.0)       # K pad (kills denom row)
            for h in range(NHC):
                nc.gpsimd.memset(ktz[h][HD:P, :], 0.0)     # K pad
                nc.gpsimd.memset(qtz[h][HD:P, :], 0.0)
                nc.gpsimd.memset(ao[h][HD:P, :], 0.0)  # K pad for outproj

            # ---- phase 1+2: qkv projection + rope + v pack + transposes ----
            with (
                tc.tile_pool(name="p1ps", bufs=3, space="PSUM") as pp,
                tc.tile_pool(name="p2ps", bufs=2, space="PSUM") as p2,
                tc.tile_pool(name="p1t", bufs=3) as tp,
            ):
                for s in range(NS):
                    pqkv = pp.tile([P, QKV], F32, tag="pqkv")
                    xsl = xt_sb[:, :, s * P : (s + 1) * P]
                    for c in range(NCC):
                        st, sp = (c == 0), (c == NCC - 1)
                        nc.tensor.matmul(
                            pqkv[:, 0:512], xsl[:, c, :], wqkv_sb[:, c, 0:512],
                            start=st, stop=sp,
                        )
                        nc.tensor.matmul(
                            pqkv[:, 512:QKV], xsl[:, c, :], wqkv_sb[:, c, 512:QKV],
                            start=st, stop=sp,
                        )
                    # rope on q and k together ([:, 0:384] of the psum tile)
                    qk = pqkv[:, 0:384]
                    qk3 = qk.rearrange("p (b x) -> p b x", x=32)
                    c3 = cos_sb[:, s, :]
                    s3 = sin_sb[:, s, :].rearrange("p (b x) -> p b x", x=32)
                    t = tp.tile([P, 384], F32, tag="ropet")
                    t3 = t.rearrange("p (b x) -> p b x", x=32)
                    # t = shuffle(qk) * sin_signed  (swap 16-halves per 32-block)
                    nc.vector.tensor_tensor(
                        t3[:, :, 0:16], qk3[:, :, 16:32], s3[:, :, 0:16], ALU.mult
                    )
                    nc.vector.tensor_tensor(
                        t3[:, :, 16:32], qk3[:, :, 0:16], s3[:, :, 16:32], ALU.mult
                    )
                    t2 = tp.tile([P, 384], F32, tag="ropet2")
                    nc.vector.tensor_tensor(t2[:], qk[:], c3[:, 0:384], ALU.mult)
                    # final add on gpsimd (sbuf-only engine), cast to bf16
                    nc.gpsimd.tensor_tensor(
                        q_sb[:, s, :], t2[:, 0:192], t[:, 0:192], ALU.add
                    )
                    nc.gpsimd.tensor_tensor(
                        k_sb[:, s, :], t2[:, 192:384], t[:, 192:384], ALU.add
                    )
                    # v pack with ones columns
                    nc.scalar.copy(
                        v_sb[:, s, :, 0:HD],
                        pqkv[:, 384:QKV].rearrange("p (h x) -> p h x", x=HD),
                    )
                    # transposes to d-major, per-head zero-padded layout
                    sl = slice(s * P, (s + 1) * P)
                    for src_t, dst, ev in ((k_sb, ktz, "v"), (q_sb, qtz, "a")):
                        pt = p2.tile([P, P], BF, tag="pt")
                        nc.tensor.transpose(pt[:], src_t[:, s, 0:P], id_sb[:])
                        pt2 = p2.tile([P, P], BF, tag="pt")
                        nc.tensor.transpose(
                            pt2[0:HD, :], src_t[:, s, P : P + HD], id_sb[:]
                        )
                        eng = nc.scalar.copy if ev == "a" else nc.vector.tensor_copy
                        eng(dst[0][0:HD, sl], pt[0:HD, :])
                        eng(dst[1][0:HD, sl], pt[HD:P, :])
                        eng(dst[2][0:HD, sl], pt2[0:HD, :])

            # ---- phase 3: attention (transposed scores) ----
            with (
                tc.tile_pool(name="scps", bufs=5, space="PSUM") as scp,
                tc.tile_pool(name="aops", bufs=2, space="PSUM") as aop,
                tc.tile_pool(name="denps", bufs=1, space="PSUM") as dnp,
                tc.tile_pool(name="expp", bufs=8) as expp,
            ):
                den_ps = dnp.tile([P, NHC, NS, 2], BF)  # pad: psum needs 4B align
                for h in range(NHC):
                    kth, qth = ktz[h], qtz[h]
                    for qj in range(NQ):
                        qsl = slice(qj * SQT, (qj + 1) * SQT)
                        pa = aop.tile([P, SQT], F32, tag="pa")
                        nki = KPQ * qj + KPQ
                        for ki in range(nki):
                            r = ki - KPQ * qj  # >= 0: diagonal-crossing tile
                            off = max(r, 0) * P
                            ps = scp.tile([P, SQT], F32, tag="ps")
                            nc.tensor.matmul(
                                ps[:, off:SQT],
                                kth[:, ki * P : (ki + 1) * P],
                                qth[:, qj * SQT + off : (qj + 1) * SQT],
                                start=True, stop=True,
                            )
                            e = expp.tile([P, SQT], BF, tag="e")
                            nc.scalar.activation(
                                e[:, off:SQT], ps[:, off:SQT], AF.Exp, scale=0.125
                            )
                            if r >= 0:
                                if off > 0:
                                    nc.vector.memset(e[:, 0:off], 0.0)
                                nc.vector.tensor_tensor(
                                    e[:, off : off + P],
                                    e[:, off : off + P],
                                    mask_sb[:],
                                    ALU.mult,
                                )
                            nc.tensor.matmul(
                                pa[:],
                                v_sb[:, ki, h, :],
                                e[:],
                                start=(ki == 0), stop=(ki == nki - 1),
                            )
                        nc.scalar.copy(ao[h][0 : HD + 1, qsl], pa[0 : HD + 1, :])
                        # denominator row -> s-major columns (tiny PE transposes)
                        for c4 in range(KPQ):
                            col = qj * KPQ + c4
                            nc.tensor.transpose(
                                den_ps[:, h, col : col + 1, 0],
                                ao[h][HD : HD + 1, col * P : (col + 1) * P],
                                id_sb[HD : HD + 1, HD : HD + 1],
                            )
                nc.vector.reciprocal(den_rec[:], den_ps[:, :, :, 0])

            # ---- phase 4: per-head output projection + normalize ----
            with (
                tc.tile_pool(name="p4ps", bufs=3, space="PSUM") as p4,
                tc.tile_pool(name="outp", bufs=3) as op,
            ):
                for s in range(NS):
                    sl = slice(s * P, (s + 1) * P)
                    acc = op.tile([P, DM], F32, tag="acc")
                    for h in range(NHC):
                        po = p4.tile([P, DM], F32, tag="po")
                        lh = ao[h][:, sl]  # K=128: denom row killed by wo zeros
                        nc.tensor.matmul(
                            po[:, 0:512], lh, wo_sb[:, h, 0:512], start=True, stop=True
                        )
                        nc.tensor.matmul(
                            po[:, 512:DM], lh, wo_sb[:, h, 512:DM], start=True, stop=True
                        )
                        scale = den_rec[:, h, s : s + 1]
                        if h == 0:
                            nc.scalar.activation(acc[:], po[:], AF.Copy, scale=scale)
                        else:
                            nc.vector.scalar_tensor_tensor(
                                acc[:], po[:], scale, acc[:], ALU.mult, ALU.add
                            )
                    nc.sync.dma_start(out_d[sl, :], acc[:])

    nc.compile()
    return nc


_cache = {}
LAST_RESULT = None


def _get_program(S, n_devices):
    key = (S, n_devices)
    if key not in _cache:
        _cache[key] = build_program(S, n_devices)
    return _cache[key]


def _rope_tables(row_ids, col_ids, S):
    inv = 1.0 / (10000.0 ** (np.arange(0, 32, 2, dtype=np.float64) / 32.0))

    def block(ids):
        ang = ids.astype(np.float64)[:, None] * inv[None, :]
        c = np.concatenate([np.cos(ang), np.cos(ang)], -1)
        s_ = np.concatenate([-np.sin(ang), np.sin(ang)], -1)  # signed (shuffle form)
        return c, s_

    cr, sr = block(np.asarray(row_ids))
    cc, sc = block(np.asarray(col_ids))
    cos64 = np.concatenate([cr, cc], -1)
    sin64 = np.concatenate([sr, sc], -1)
    return (
        np.tile(cos64, (1, 2 * NHC)).astype(bfloat16),
        np.tile(sin64, (1, 2 * NHC)).astype(bfloat16),
    )


def _make_masks():
    pp_ = np.arange(P)[:, None]
    ff = np.arange(P)[None, :]
    return (pp_ <= ff).astype(np.float32).astype(bfloat16)


def kernel(x, row_ids, col_ids, Wq, Wk, Wv, Wo):
    x = np.asarray(x)
    B, S, _ = x.shape
    n_cores = 8
    groups = n_cores // B  # head groups per batch (4)
    hpg = NHC  # heads per group

    nc = _get_program(S, n_cores)
    cos_t, sin_t = _rope_tables(row_ids, col_ids, S)
    masks = _make_masks()
    ident = np.eye(P, dtype=bfloat16)

    Wq, Wk, Wv, Wo = (np.asarray(w, np.float32) for w in (Wq, Wk, Wv, Wo))
    in_maps = []
    for c in range(n_cores):
        b = c // groups
        h0 = (c % groups) * hpg
        rows = slice(h0 * HD, (h0 + hpg) * HD)
        xt = np.ascontiguousarray(x[b].T).astype(bfloat16)
        wqkv = np.concatenate(
            [Wq[rows].T, Wk[rows].T, Wv[rows].T], axis=1
        ).astype(bfloat16)
        wo = np.ascontiguousarray(Wo[:, rows].T).astype(bfloat16)
        in_maps.append(
            {
                "xt": xt,
                "wqkv": wqkv,
                "wo": wo,
                "cos": cos_t,
                "sin": sin_t,
                "masks": masks,
                "ident": ident,
            }
        )

    import os

    trace = bool(os.environ.get("KERNEL_TRACE"))
    kw = {}
    if trace and os.environ.get("KERNEL_TRACE_DIR"):
        kw["tmpdir"] = os.environ["KERNEL_TRACE_DIR"]
    res = run_bass_kernel_spmd(nc, in_maps, list(range(n_cores)), trace=trace, **kw)
    global LAST_RESULT
    LAST_RESULT = res

    outs = [res.results[c]["outp"] for c in range(n_cores)]
    out = np.stack(
        [sum(outs[b * groups + g] for g in range(groups)) for b in range(B)], axis=0
    )
    return out.astype(np.float32)
